# revision 14
# baseline (speedup 1.0000x reference)
"""Trainium2 Bass/Tile kernel: two chained VALID 3x3 convolutions.

    x  [N,3,256,256] --conv(w1)--> h [N,64,254,254] --conv(w2)--> out [N,128,252,252]

Data-parallel over 8 NeuronCores: batch N=16 -> 2 images per core, conv
weights replicated.  Per core the convs are computed as implicit GEMMs on the
tensor engine (PE observed pinned at the 1.2 GHz throttled clock in this
environment, ~420 ns per 504-column bf16 matmul, so the win is fewer/denser
passes, not HAM warmup).

  conv1: contraction over C0*3*3=27 on SBUF partitions (im2col buffer built
         with 9 strided DMAs).  Column-tiled pair of matmuls per 2-row chunk
         produces the *doubled* h layout directly in PSUM:
           partitions 0:64  <- h rows (r, r+1)     (tile_position (0,0))
           partitions 64:128<- h rows (r+1, r+2)   (tile_position (0,64))
         so no SBUF->SBUF row-shift DMA is needed.

  conv2: contraction over C1*9=576 = 4.5 x 128.  Per output row-pair tile:
         3 K=128 matmuls cover taps (0,dj)+(1,dj) using the doubled H.
         The leftover taps (2,dj) are K=64 singles; singles of TWO adjacent
         output tiles are row-group-packed (tile_position rows 0 vs 64) so
         they run concurrently in the PE array: 9 effective passes per two
         tiles instead of 12.  PSUM accumulates; DVE copies to SBUF; DMA out.

MODE "bf16": inputs cast to bfloat16 host-side, fp32 PSUM accumulation
(measured scale-rel absmax err ~3.5e-3).
"""

from contextlib import ExitStack

import ml_dtypes
import numpy as np

import concourse.bass as bass
import concourse.mybir as mybir
import concourse.tile as tile
import concourse.bass_utils as bass_utils
from concourse import bacc

N_CORES = 8
FULL_N = 16
C0, C1, C2 = 3, 64, 128

MODE = "bf16"


def _mm_dt():
    return mybir.dt.bfloat16 if MODE == "bf16" else mybir.dt.float32r


def _np_dt():
    return ml_dtypes.bfloat16 if MODE == "bf16" else np.float32


class Geom:
    def __init__(self, npc, h0, w0, ty):
        self.npc = npc          # images per core
        self.h0, self.w0 = h0, w0
        self.h1, self.w1 = h0 - 2, w0 - 2
        self.h2, self.w2 = h0 - 4, w0 - 4
        self.ty = ty            # conv2 output rows per strip
        assert ty % 4 == 0 and self.h2 % ty == 0


GEOM = Geom(npc=FULL_N // N_CORES, h0=256, w0=256, ty=28)


def _emit(ctx: ExitStack, tc: tile.TileContext, g: Geom, out, x, w1t, w2p, w2s2,
          mm_dt):
    nc = tc.nc
    f32 = mybir.dt.float32
    TY, W1, W2 = g.ty, g.w1, g.w2

    wpool = ctx.enter_context(tc.tile_pool(name="weights", bufs=1))
    b1pool = ctx.enter_context(tc.tile_pool(name="b1", bufs=3))
    hpool = ctx.enter_context(tc.tile_pool(name="h", bufs=2))
    opool = ctx.enter_context(tc.tile_pool(name="o2", bufs=4))
    ps1 = ctx.enter_context(tc.tile_pool(name="ps1", bufs=4, space="PSUM"))
    ps2 = ctx.enter_context(tc.tile_pool(name="ps2", bufs=4, space="PSUM"))

    w1t_sb = wpool.tile([27, C1], mm_dt)
    nc.sync.dma_start(w1t_sb[:], w1t)
    w2p_sb = wpool.tile([128, 3, C2], mm_dt)
    w2s_sb = wpool.tile([128, 3, C2], mm_dt)

    def load_b1(n, y0, spread=False):
        """Issue the 9 im2col DMAs for strip (n, y0); returns the tile."""
        # partition (di*3+dj)*3+c holds x[c, y0+r+di, dj:dj+W1]
        B1 = b1pool.tile([27, TY + 2, W1], mm_dt, tag="b1")
        engs = (nc.sync, nc.gpsimd, nc.scalar) if spread else (nc.sync,)
        for t9 in range(9):
            di, dj = divmod(t9, 3)
            engs[t9 % len(engs)].dma_start(
                B1[3 * t9:3 * t9 + 3],
                x[n, :, y0 + di:y0 + di + TY + 2, dj:dj + W1])
        return B1

    def alloc_h():
        return hpool.tile([128, TY + 2, W1], mm_dt, tag="h", name="H")

    def conv1_slot(B1, H, r, copy_eng):
        """Produce doubled h rows: top = (r, r+1), bottom = (r+1, r+2)."""
        last = r + 2 >= TY + 2  # bottom half would run past the strip
        P1 = ps1.tile([128, 2, W1], f32, tag="p1")
        # col group 0: h rows r, r+1 -> PSUM partitions 0:64
        nc.tensor.matmul(P1[0:C1], w1t_sb[:], B1[:, r:r + 2, :],
                         start=True, stop=True)
        if not last:
            # col group 64: h rows r+1, r+2 -> PSUM partitions 64:128
            nc.tensor.matmul(P1[C1:128], w1t_sb[:], B1[:, r + 1:r + 3, :],
                             start=True, stop=True)
            copy_eng(H[:, r:r + 2, :], P1[:])
        else:
            copy_eng(H[0:C1, r:r + 2, :], P1[0:C1])

    def conv2_pair(n, y0, H, tA, mid_hook=None):
        """Two output row-pair tiles (tA, tB=tA+2): 6 K=128 pair matmuls +
        6 K=64 singles packed two-at-a-time into PE row groups 0/64.
        mid_hook() is called between the pair blocks to interleave conv1
        slots finely (keeps copy engines evenly loaded, PE dense)."""
        tB = tA + 2
        PA = ps2.tile([C2, 2, W2], f32, tag="p2")
        PB = ps2.tile([C2, 2, W2], f32, tag="p2")
        for dj in range(3):  # taps (0,dj)+(1,dj) for tile A
            nc.tensor.matmul(PA[:], w2p_sb[:, dj, :],
                             H[:, tA:tA + 2, dj:dj + W2],
                             start=(dj == 0), stop=False)
        if mid_hook is not None:
            mid_hook()
        for dj in range(3):  # taps (0,dj)+(1,dj) for tile B
            nc.tensor.matmul(PB[:], w2p_sb[:, dj, :],
                             H[:, tB:tB + 2, dj:dj + W2],
                             start=(dj == 0), stop=False)
        if mid_hook is not None:
            mid_hook()
        # singles: tap (2,dj).  Top half (rows r = h row r) serves tile A,
        # bottom half (rows r = h row r+1) serves tile B, concurrently.
        # Bottom rows only span 0..TY-1, so the last tile-pair flips:
        # B reads top (needs h row TY+1), A reads bottom.
        b_on_top = tB + 2 > TY - 1  # B's bottom rows tB+1..tB+2 out of range
        for dj in range(3):
            stop = dj == 2
            if not b_on_top:
                nc.tensor.matmul(PA[:], w2s_sb[0:C1, dj, :],
                                 H[0:C1, tA + 2:tA + 4, dj:dj + W2],
                                 start=False, stop=stop)
                nc.tensor.matmul(PB[:], w2s_sb[C1:128, dj, :],
                                 H[C1:128, tB + 1:tB + 3, dj:dj + W2],
                                 start=False, stop=stop)
            else:
                nc.tensor.matmul(PA[:], w2s_sb[C1:128, dj, :],
                                 H[C1:128, tA + 1:tA + 3, dj:dj + W2],
                                 start=False, stop=stop)
                nc.tensor.matmul(PB[:], w2s_sb[0:C1, dj, :],
                                 H[0:C1, tB + 2:tB + 4, dj:dj + W2],
                                 start=False, stop=stop)
        # split the PSUM->SBUF copies across both PSUM-capable engines and
        # the out-DMAs across two queues so neither serializes the PE
        for t, P, copy_eng, dma_eng in (
                (tA, PA, nc.vector.tensor_copy, nc.gpsimd),
                (tB, PB, nc.scalar.copy, nc.sync)):
            O2 = opool.tile([C2, 2, W2], f32, tag="o2")
            copy_eng(O2[:], P[:])
            dma_eng.dma_start(out[n, :, y0 + t:y0 + t + 2, :], O2[:])

    strips = [(n, y0) for n in range(g.npc) for y0 in range(0, g.h2, TY)]
    n_c1 = (TY + 2 + 1) // 2          # conv1 slots per strip (r = 0,2..TY)
    # software pipeline, im2col prefetched a full strip early:
    #   strip s body: issue B1(s+2) DMAs, conv2(s) interleaved with conv1(s+1)
    # so B1(s+1) had all of strip s-1's compute (~17us) to stream in.
    B1 = {0: load_b1(*strips[0], spread=True)}
    if len(strips) > 1:
        B1[1] = load_b1(*strips[1])
    # conv2 weights can land any time before conv2(0); off the critical queue
    nc.gpsimd.dma_start(w2p_sb[:], w2p)
    nc.gpsimd.dma_start(w2s_sb[:], w2s2)
    Hcur = alloc_h()
    for r in range(0, TY + 2, 2):
        eng = nc.vector.tensor_copy if (r // 2) % 2 else nc.scalar.copy
        conv1_slot(B1[0], Hcur, r, eng)
    for i, (n, y0) in enumerate(strips):
        nxt = strips[i + 1] if i + 1 < len(strips) else None
        if i + 2 < len(strips):
            B1[i + 2] = load_b1(*strips[i + 2])
        if nxt is not None:
            Hnxt = alloc_h()
        state = {"r": 0}

        def c1_hook(budget=1):
            # one conv1 slot per call point; 15 slots spread over the 14
            # hook points of 7 tile-pairs (the first hook emits two)
            k = 0
            while state["r"] < n_c1 and k < budget:
                rr = state["r"]
                eng = nc.vector.tensor_copy if rr % 2 else nc.scalar.copy
                conv1_slot(B1[i + 1], Hnxt, 2 * rr, eng)
                state["r"] += 1
                k += 1

        for tA in range(0, TY, 4):
            hook = None
            if nxt is not None:
                left = n_c1 - state["r"]
                hooks_left = 2 * (TY - tA) // 4
                hook = (lambda: c1_hook(2)) if left > hooks_left else c1_hook
            conv2_pair(n, y0, Hcur, tA, mid_hook=hook)
        B1.pop(i, None)
        if nxt is not None:
            Hcur = Hnxt


def build(g: Geom = GEOM, mm_dt=None):
    if mm_dt is None:
        mm_dt = _mm_dt()
    nc = bacc.Bacc("TRN2", target_bir_lowering=False, debug=False,
                   num_devices=N_CORES)
    f32 = mybir.dt.float32
    x = nc.dram_tensor("x", [g.npc, C0, g.h0, g.w0], mm_dt,
                       kind="ExternalInput").ap()
    w1t = nc.dram_tensor("w1t", [27, C1], mm_dt, kind="ExternalInput").ap()
    w2p = nc.dram_tensor("w2p", [128, 3, C2], mm_dt, kind="ExternalInput").ap()
    w2s2 = nc.dram_tensor("w2s2", [128, 3, C2], mm_dt,
                          kind="ExternalInput").ap()
    out = nc.dram_tensor("out", [g.npc, C2, g.h2, g.w2], f32,
                         kind="ExternalOutput").ap()
    with tile.TileContext(nc) as tc:
        with ExitStack() as ctx:
            _emit(ctx, tc, g, out, x, w1t, w2p, w2s2, mm_dt)
    nc.compile()
    return nc


def host_round(a: np.ndarray) -> np.ndarray:
    """Cast fp32 to the matmul storage dtype (bf16 cast, or tf32 rounding)."""
    a = np.ascontiguousarray(a, dtype=np.float32)
    if MODE == "bf16":
        return a.astype(ml_dtypes.bfloat16)
    b = a.view(np.uint32).copy()
    b += 0xFFF + ((b >> 13) & 1)
    b &= np.uint32(0xFFFFE000)
    return b.view(np.float32)


def pack_weights(w1: np.ndarray, w2: np.ndarray):
    """Host-side repack so every device DMA is contiguous.

    w1t[p, o] = w1[o, c, di, dj] with p = (di*3+dj)*3 + c  (matches im2col)
    w2p[k, dj, o]: k<64 -> w2[o, k, 0, dj]; k>=64 -> w2[o, k-64, 1, dj]
    w2s2[k, dj, o] = w2[o, k mod 64, 2, dj]  (tap-2 weights, both halves)
    """
    w1 = np.ascontiguousarray(np.asarray(w1), dtype=np.float32)
    w2 = np.ascontiguousarray(np.asarray(w2), dtype=np.float32)
    w1t = np.ascontiguousarray(w1.transpose(2, 3, 1, 0).reshape(27, C1))
    w2p = np.empty((128, 3, C2), np.float32)
    w2p[:C1] = w2[:, :, 0, :].transpose(1, 2, 0)
    w2p[C1:] = w2[:, :, 1, :].transpose(1, 2, 0)
    w2s = w2[:, :, 2, :].transpose(1, 2, 0)
    w2s2 = np.ascontiguousarray(np.concatenate([w2s, w2s], axis=0))
    return host_round(w1t), host_round(w2p), host_round(w2s2)


_NC_CACHE: dict = {}


def _get_nc():
    key = ("main", MODE)
    if key not in _NC_CACHE:
        _NC_CACHE[key] = build()
    return _NC_CACHE[key]


def run(x, w1, w2, trace: bool = False):
    """Shard, run on 8 cores, gather.  Returns (out, BassKernelResults)."""
    x = np.ascontiguousarray(np.asarray(x), dtype=np.float32)
    assert x.shape == (FULL_N, C0, GEOM.h0, GEOM.w0), x.shape
    w1t, w2p, w2s2 = pack_weights(w1, w2)
    xs = host_round(x).reshape(N_CORES, GEOM.npc, C0, GEOM.h0, GEOM.w0)
    in_maps = [
        {"x": np.ascontiguousarray(xs[c]), "w1t": w1t, "w2p": w2p,
         "w2s2": w2s2}
        for c in range(N_CORES)
    ]
    nc = _get_nc()
    res = bass_utils.run_bass_kernel_spmd(
        nc, in_maps, core_ids=list(range(N_CORES)), trace=trace)
    out = np.concatenate([r["out"] for r in res.results], axis=0)
    return out, res


def kernel(x, w1, w2):
    out, _ = run(x, w1, w2, trace=False)
    return out


# revision 15
# speedup vs baseline: 1.0807x; 1.0807x over previous
"""Trainium2 Bass/Tile kernel: two chained VALID 3x3 convolutions.

    x  [N,3,256,256] --conv(w1)--> h [N,64,254,254] --conv(w2)--> out [N,128,252,252]

Data-parallel over 8 NeuronCores: batch N=16 -> 2 images per core, conv
weights replicated.  Per core the convs are computed as implicit GEMMs on the
tensor engine (PE observed pinned at the 1.2 GHz throttled clock in this
environment, ~420 ns per 504-column bf16 matmul, so the win is fewer/denser
passes, not HAM warmup).

  conv1: contraction over C0*3*3=27 on SBUF partitions (im2col buffer built
         with 9 strided DMAs).  Column-tiled pair of matmuls per 2-row chunk
         produces the *doubled* h layout directly in PSUM:
           partitions 0:64  <- h rows (r, r+1)     (tile_position (0,0))
           partitions 64:128<- h rows (r+1, r+2)   (tile_position (0,64))
         so no SBUF->SBUF row-shift DMA is needed.

  conv2: contraction over C1*9=576 = 4.5 x 128.  Per output row-pair tile:
         3 K=128 matmuls cover taps (0,dj)+(1,dj) using the doubled H.
         The leftover taps (2,dj) are K=64 singles; singles of TWO adjacent
         output tiles are row-group-packed (tile_position rows 0 vs 64) so
         they run concurrently in the PE array: 9 effective passes per two
         tiles instead of 12.  PSUM accumulates; DVE copies to SBUF; DMA out.

MODE "bf16": inputs cast to bfloat16 host-side, fp32 PSUM accumulation
(measured scale-rel absmax err ~3.5e-3).
"""

from contextlib import ExitStack

import ml_dtypes
import numpy as np

import concourse.bass as bass
import concourse.mybir as mybir
import concourse.tile as tile
import concourse.bass_utils as bass_utils
from concourse import bacc

N_CORES = 8
FULL_N = 16
C0, C1, C2 = 3, 64, 128

MODE = "bf16"


def _mm_dt():
    return mybir.dt.bfloat16 if MODE == "bf16" else mybir.dt.float32r


def _np_dt():
    return ml_dtypes.bfloat16 if MODE == "bf16" else np.float32


class Geom:
    def __init__(self, npc, h0, w0, ty):
        self.npc = npc          # images per core
        self.h0, self.w0 = h0, w0
        self.h1, self.w1 = h0 - 2, w0 - 2
        self.h2, self.w2 = h0 - 4, w0 - 4
        self.ty = ty            # conv2 output rows per strip
        assert ty % 4 == 0 and self.h2 % ty == 0


GEOM = Geom(npc=FULL_N // N_CORES, h0=256, w0=256, ty=28)


def _emit(ctx: ExitStack, tc: tile.TileContext, g: Geom, out, x, w1t, w2p, w2s2,
          mm_dt):
    nc = tc.nc
    f32 = mybir.dt.float32
    TY, W1, W2 = g.ty, g.w1, g.w2

    wpool = ctx.enter_context(tc.tile_pool(name="weights", bufs=1))
    b1pool = ctx.enter_context(tc.tile_pool(name="b1", bufs=3))
    hpool = ctx.enter_context(tc.tile_pool(name="h", bufs=2))
    opool = ctx.enter_context(tc.tile_pool(name="o2", bufs=4))
    ps1 = ctx.enter_context(tc.tile_pool(name="ps1", bufs=4, space="PSUM"))
    ps2 = ctx.enter_context(tc.tile_pool(name="ps2", bufs=4, space="PSUM"))

    w1t_sb = wpool.tile([27, C1], mm_dt)
    nc.sync.dma_start(w1t_sb[:], w1t)
    w2p_sb = wpool.tile([128, 3, C2], mm_dt)
    w2s_sb = wpool.tile([128, 3, C2], mm_dt)

    def load_b1(n, y0, spread=False):
        """Issue the 9 im2col DMAs for strip (n, y0); returns the tile."""
        # partition (di*3+dj)*3+c holds x[c, y0+r+di, dj:dj+W1]
        B1 = b1pool.tile([27, TY + 2, W1], mm_dt, tag="b1")
        engs = (nc.sync, nc.gpsimd, nc.scalar) if spread else (nc.sync,)
        for t9 in range(9):
            di, dj = divmod(t9, 3)
            engs[t9 % len(engs)].dma_start(
                B1[3 * t9:3 * t9 + 3],
                x[n, :, y0 + di:y0 + di + TY + 2, dj:dj + W1])
        return B1

    def alloc_h():
        return hpool.tile([128, TY + 2, W1], mm_dt, tag="h", name="H")

    def conv1_slot(B1, H, r, copy_eng):
        """Produce doubled h rows: top = (r, r+1), bottom = (r+1, r+2)."""
        last = r + 2 >= TY + 2  # bottom half would run past the strip
        P1 = ps1.tile([128, 2, W1], f32, tag="p1")
        # col group 0: h rows r, r+1 -> PSUM partitions 0:64
        nc.tensor.matmul(P1[0:C1], w1t_sb[:], B1[:, r:r + 2, :],
                         start=True, stop=True)
        if not last:
            # col group 64: h rows r+1, r+2 -> PSUM partitions 64:128
            nc.tensor.matmul(P1[C1:128], w1t_sb[:], B1[:, r + 1:r + 3, :],
                             start=True, stop=True)
            copy_eng(H[:, r:r + 2, :], P1[:])
        else:
            copy_eng(H[0:C1, r:r + 2, :], P1[0:C1])

    def conv2_pair(n, y0, H, tA, mid_hook=None):
        """Two output row-pair tiles (tA, tB=tA+2): 6 K=128 pair matmuls +
        6 K=64 singles packed two-at-a-time into PE row groups 0/64.
        mid_hook() is called between the pair blocks to interleave conv1
        slots finely (keeps copy engines evenly loaded, PE dense)."""
        tB = tA + 2
        PA = ps2.tile([C2, 2, W2], f32, tag="p2")
        PB = ps2.tile([C2, 2, W2], f32, tag="p2")
        for dj in range(3):  # taps (0,dj)+(1,dj) for tile A
            nc.tensor.matmul(PA[:], w2p_sb[:, dj, :],
                             H[:, tA:tA + 2, dj:dj + W2],
                             start=(dj == 0), stop=False)
        if mid_hook is not None:
            mid_hook()
        for dj in range(3):  # taps (0,dj)+(1,dj) for tile B
            nc.tensor.matmul(PB[:], w2p_sb[:, dj, :],
                             H[:, tB:tB + 2, dj:dj + W2],
                             start=(dj == 0), stop=False)
        if mid_hook is not None:
            mid_hook()
        # singles: tap (2,dj).  Top half (rows r = h row r) serves tile A,
        # bottom half (rows r = h row r+1) serves tile B, concurrently.
        # Bottom rows only span 0..TY-1, so the last tile-pair flips:
        # B reads top (needs h row TY+1), A reads bottom.
        b_on_top = tB + 2 > TY - 1  # B's bottom rows tB+1..tB+2 out of range
        for dj in range(3):
            stop = dj == 2
            if not b_on_top:
                nc.tensor.matmul(PA[:], w2s_sb[0:C1, dj, :],
                                 H[0:C1, tA + 2:tA + 4, dj:dj + W2],
                                 start=False, stop=stop)
                nc.tensor.matmul(PB[:], w2s_sb[C1:128, dj, :],
                                 H[C1:128, tB + 1:tB + 3, dj:dj + W2],
                                 start=False, stop=stop)
            else:
                nc.tensor.matmul(PA[:], w2s_sb[C1:128, dj, :],
                                 H[C1:128, tA + 1:tA + 3, dj:dj + W2],
                                 start=False, stop=stop)
                nc.tensor.matmul(PB[:], w2s_sb[0:C1, dj, :],
                                 H[0:C1, tB + 2:tB + 4, dj:dj + W2],
                                 start=False, stop=stop)
        # split the PSUM->SBUF copies across both PSUM-capable engines; all
        # out-DMAs stay on the gpsimd queue (sync queue must stay clear for
        # im2col prefetch -- a waiting out-DMA head-of-line blocks it)
        for t, P, copy_eng in ((tA, PA, nc.vector.tensor_copy),
                               (tB, PB, nc.scalar.copy)):
            O2 = opool.tile([C2, 2, W2], f32, tag="o2")
            copy_eng(O2[:], P[:])
            nc.gpsimd.dma_start(out[n, :, y0 + t:y0 + t + 2, :], O2[:])

    strips = [(n, y0) for n in range(g.npc) for y0 in range(0, g.h2, TY)]
    n_c1 = (TY + 2 + 1) // 2          # conv1 slots per strip (r = 0,2..TY)
    # software pipeline, im2col prefetched a full strip early:
    #   strip s body: issue B1(s+2) DMAs, conv2(s) interleaved with conv1(s+1)
    # so B1(s+1) had all of strip s-1's compute (~17us) to stream in.
    B1 = {0: load_b1(*strips[0], spread=True)}
    if len(strips) > 1:
        B1[1] = load_b1(*strips[1])
    # conv2 weights can land any time before conv2(0); off the critical queue
    nc.gpsimd.dma_start(w2p_sb[:], w2p)
    nc.gpsimd.dma_start(w2s_sb[:], w2s2)
    Hcur = alloc_h()
    for r in range(0, TY + 2, 2):
        eng = nc.vector.tensor_copy if (r // 2) % 2 else nc.scalar.copy
        conv1_slot(B1[0], Hcur, r, eng)
    for i, (n, y0) in enumerate(strips):
        nxt = strips[i + 1] if i + 1 < len(strips) else None
        if i + 2 < len(strips):
            B1[i + 2] = load_b1(*strips[i + 2])
        if nxt is not None:
            Hnxt = alloc_h()
        state = {"r": 0}

        def c1_hook(budget=1):
            # one conv1 slot per call point; 15 slots spread over the 14
            # hook points of 7 tile-pairs (the first hook emits two)
            k = 0
            while state["r"] < n_c1 and k < budget:
                rr = state["r"]
                eng = nc.vector.tensor_copy if rr % 2 else nc.scalar.copy
                conv1_slot(B1[i + 1], Hnxt, 2 * rr, eng)
                state["r"] += 1
                k += 1

        for tA in range(0, TY, 4):
            hook = None
            if nxt is not None:
                left = n_c1 - state["r"]
                hooks_left = 2 * (TY - tA) // 4
                hook = (lambda: c1_hook(2)) if left > hooks_left else c1_hook
            conv2_pair(n, y0, Hcur, tA, mid_hook=hook)
        B1.pop(i, None)
        if nxt is not None:
            Hcur = Hnxt


def build(g: Geom = GEOM, mm_dt=None):
    if mm_dt is None:
        mm_dt = _mm_dt()
    nc = bacc.Bacc("TRN2", target_bir_lowering=False, debug=False,
                   num_devices=N_CORES)
    f32 = mybir.dt.float32
    x = nc.dram_tensor("x", [g.npc, C0, g.h0, g.w0], mm_dt,
                       kind="ExternalInput").ap()
    w1t = nc.dram_tensor("w1t", [27, C1], mm_dt, kind="ExternalInput").ap()
    w2p = nc.dram_tensor("w2p", [128, 3, C2], mm_dt, kind="ExternalInput").ap()
    w2s2 = nc.dram_tensor("w2s2", [128, 3, C2], mm_dt,
                          kind="ExternalInput").ap()
    out = nc.dram_tensor("out", [g.npc, C2, g.h2, g.w2], f32,
                         kind="ExternalOutput").ap()
    with tile.TileContext(nc) as tc:
        with ExitStack() as ctx:
            _emit(ctx, tc, g, out, x, w1t, w2p, w2s2, mm_dt)
    nc.compile()
    return nc


def host_round(a: np.ndarray) -> np.ndarray:
    """Cast fp32 to the matmul storage dtype (bf16 cast, or tf32 rounding)."""
    a = np.ascontiguousarray(a, dtype=np.float32)
    if MODE == "bf16":
        return a.astype(ml_dtypes.bfloat16)
    b = a.view(np.uint32).copy()
    b += 0xFFF + ((b >> 13) & 1)
    b &= np.uint32(0xFFFFE000)
    return b.view(np.float32)


def pack_weights(w1: np.ndarray, w2: np.ndarray):
    """Host-side repack so every device DMA is contiguous.

    w1t[p, o] = w1[o, c, di, dj] with p = (di*3+dj)*3 + c  (matches im2col)
    w2p[k, dj, o]: k<64 -> w2[o, k, 0, dj]; k>=64 -> w2[o, k-64, 1, dj]
    w2s2[k, dj, o] = w2[o, k mod 64, 2, dj]  (tap-2 weights, both halves)
    """
    w1 = np.ascontiguousarray(np.asarray(w1), dtype=np.float32)
    w2 = np.ascontiguousarray(np.asarray(w2), dtype=np.float32)
    w1t = np.ascontiguousarray(w1.transpose(2, 3, 1, 0).reshape(27, C1))
    w2p = np.empty((128, 3, C2), np.float32)
    w2p[:C1] = w2[:, :, 0, :].transpose(1, 2, 0)
    w2p[C1:] = w2[:, :, 1, :].transpose(1, 2, 0)
    w2s = w2[:, :, 2, :].transpose(1, 2, 0)
    w2s2 = np.ascontiguousarray(np.concatenate([w2s, w2s], axis=0))
    return host_round(w1t), host_round(w2p), host_round(w2s2)


_NC_CACHE: dict = {}


def _get_nc():
    key = ("main", MODE)
    if key not in _NC_CACHE:
        _NC_CACHE[key] = build()
    return _NC_CACHE[key]


def run(x, w1, w2, trace: bool = False):
    """Shard, run on 8 cores, gather.  Returns (out, BassKernelResults)."""
    x = np.ascontiguousarray(np.asarray(x), dtype=np.float32)
    assert x.shape == (FULL_N, C0, GEOM.h0, GEOM.w0), x.shape
    w1t, w2p, w2s2 = pack_weights(w1, w2)
    xs = host_round(x).reshape(N_CORES, GEOM.npc, C0, GEOM.h0, GEOM.w0)
    in_maps = [
        {"x": np.ascontiguousarray(xs[c]), "w1t": w1t, "w2p": w2p,
         "w2s2": w2s2}
        for c in range(N_CORES)
    ]
    nc = _get_nc()
    res = bass_utils.run_bass_kernel_spmd(
        nc, in_maps, core_ids=list(range(N_CORES)), trace=trace)
    out = np.concatenate([r["out"] for r in res.results], axis=0)
    return out, res


def kernel(x, w1, w2):
    out, _ = run(x, w1, w2, trace=False)
    return out


# revision 19
# speedup vs baseline: 1.0912x; 1.0097x over previous
"""Trainium2 Bass/Tile kernel: two chained VALID 3x3 convolutions.

    x  [N,3,256,256] --conv(w1)--> h [N,64,254,254] --conv(w2)--> out [N,128,252,252]

Data-parallel over 8 NeuronCores: batch N=16 -> 2 images per core, conv
weights replicated.  Per core the convs are computed as implicit GEMMs on the
tensor engine (PE observed pinned at the 1.2 GHz throttled clock in this
environment, ~420 ns per 504-column bf16 matmul, so the win is fewer/denser
passes, not HAM warmup).

  conv1: contraction over C0*3*3=27 on SBUF partitions (im2col buffer built
         with 9 strided DMAs).  Column-tiled pair of matmuls per 2-row chunk
         produces the *doubled* h layout directly in PSUM:
           partitions 0:64  <- h rows (r, r+1)     (tile_position (0,0))
           partitions 64:128<- h rows (r+1, r+2)   (tile_position (0,64))
         so no SBUF->SBUF row-shift DMA is needed.

  conv2: contraction over C1*9=576 = 4.5 x 128.  Per output row-pair tile:
         3 K=128 matmuls cover taps (0,dj)+(1,dj) using the doubled H.
         The leftover taps (2,dj) are K=64 singles; singles of TWO adjacent
         output tiles are row-group-packed (tile_position rows 0 vs 64) so
         they run concurrently in the PE array: 9 effective passes per two
         tiles instead of 12.  PSUM accumulates; DVE copies to SBUF; DMA out.

MODE "bf16": inputs cast to bfloat16 host-side, fp32 PSUM accumulation
(measured scale-rel absmax err ~3.5e-3).
"""

from contextlib import ExitStack

import ml_dtypes
import numpy as np

import concourse.bass as bass
import concourse.mybir as mybir
import concourse.tile as tile
import concourse.bass_utils as bass_utils
from concourse import bacc

N_CORES = 8
FULL_N = 16
C0, C1, C2 = 3, 64, 128

MODE = "bf16"


def _mm_dt():
    return mybir.dt.bfloat16 if MODE == "bf16" else mybir.dt.float32r


def _np_dt():
    return ml_dtypes.bfloat16 if MODE == "bf16" else np.float32


class Geom:
    def __init__(self, npc, h0, w0, ty):
        self.npc = npc          # images per core
        self.h0, self.w0 = h0, w0
        self.h1, self.w1 = h0 - 2, w0 - 2
        self.h2, self.w2 = h0 - 4, w0 - 4
        self.ty = ty            # conv2 output rows per strip
        assert ty % 4 == 0 and self.h2 % ty == 0


GEOM = Geom(npc=FULL_N // N_CORES, h0=256, w0=256, ty=28)


def _emit(ctx: ExitStack, tc: tile.TileContext, g: Geom, out, x, w1t, w2p, w2s2,
          mm_dt):
    nc = tc.nc
    f32 = mybir.dt.float32
    TY, W1, W2 = g.ty, g.w1, g.w2

    wpool = ctx.enter_context(tc.tile_pool(name="weights", bufs=1))
    b1pool = ctx.enter_context(tc.tile_pool(name="b1", bufs=3))
    hpool = ctx.enter_context(tc.tile_pool(name="h", bufs=2))
    opool = ctx.enter_context(tc.tile_pool(name="o2", bufs=4))
    ps1 = ctx.enter_context(tc.tile_pool(name="ps1", bufs=4, space="PSUM"))
    ps2 = ctx.enter_context(tc.tile_pool(name="ps2", bufs=4, space="PSUM"))

    w1t_sb = wpool.tile([27, C1], mm_dt)
    nc.sync.dma_start(w1t_sb[:], w1t)
    w2p_sb = wpool.tile([128, 3, C2], mm_dt)
    w2s_sb = wpool.tile([128, 3, C2], mm_dt)

    def load_b1(n, y0, spread=False):
        """Issue the 9 im2col DMAs for strip (n, y0); returns the tile."""
        # partition (di*3+dj)*3+c holds x[c, y0+r+di, dj:dj+W1]
        B1 = b1pool.tile([27, TY + 2, W1], mm_dt, tag="b1")
        engs = (nc.sync, nc.gpsimd, nc.scalar) if spread else (nc.sync,)
        for t9 in range(9):
            di, dj = divmod(t9, 3)
            engs[t9 % len(engs)].dma_start(
                B1[3 * t9:3 * t9 + 3],
                x[n, :, y0 + di:y0 + di + TY + 2, dj:dj + W1])
        return B1

    def alloc_h():
        return hpool.tile([128, TY + 2, W1], mm_dt, tag="h", name="H")

    def conv1_slot(B1, H, r, copy_eng):
        """Produce doubled h rows: top = (r, r+1), bottom = (r+1, r+2)."""
        last = r + 2 >= TY + 2  # bottom half would run past the strip
        P1 = ps1.tile([128, 2, W1], f32, tag="p1")
        # col group 0: h rows r, r+1 -> PSUM partitions 0:64
        nc.tensor.matmul(P1[0:C1], w1t_sb[:], B1[:, r:r + 2, :],
                         start=True, stop=True)
        if not last:
            # col group 64: h rows r+1, r+2 -> PSUM partitions 64:128
            nc.tensor.matmul(P1[C1:128], w1t_sb[:], B1[:, r + 1:r + 3, :],
                             start=True, stop=True)
            copy_eng(H[:, r:r + 2, :], P1[:])
        else:
            copy_eng(H[0:C1, r:r + 2, :], P1[0:C1])

    def conv2_pair(n, y0, H, tA, mid_hook=None):
        """Two output row-pair tiles (tA, tB=tA+2): 6 K=128 pair matmuls +
        6 K=64 singles packed two-at-a-time into PE row groups 0/64.
        mid_hook() is called between the pair blocks to interleave conv1
        slots finely (keeps copy engines evenly loaded, PE dense)."""
        tB = tA + 2
        PA = ps2.tile([C2, 2, W2], f32, tag="p2")
        PB = ps2.tile([C2, 2, W2], f32, tag="p2")
        for dj in range(3):  # taps (0,dj)+(1,dj) for tile A
            nc.tensor.matmul(PA[:], w2p_sb[:, dj, :],
                             H[:, tA:tA + 2, dj:dj + W2],
                             start=(dj == 0), stop=False)
        if mid_hook is not None:
            mid_hook(1)
        for dj in range(3):  # taps (0,dj)+(1,dj) for tile B
            nc.tensor.matmul(PB[:], w2p_sb[:, dj, :],
                             H[:, tB:tB + 2, dj:dj + W2],
                             start=(dj == 0), stop=False)
        if mid_hook is not None:
            mid_hook(2)
        # singles: tap (2,dj).  Top half (rows r = h row r) serves tile A,
        # bottom half (rows r = h row r+1) serves tile B, concurrently.
        # Bottom rows only span 0..TY-1, so the last tile-pair flips:
        # B reads top (needs h row TY+1), A reads bottom.
        b_on_top = tB + 2 > TY - 1  # B's bottom rows tB+1..tB+2 out of range
        for dj in range(3):
            stop = dj == 2
            if not b_on_top:
                nc.tensor.matmul(PA[:], w2s_sb[0:C1, dj, :],
                                 H[0:C1, tA + 2:tA + 4, dj:dj + W2],
                                 start=False, stop=stop)
                nc.tensor.matmul(PB[:], w2s_sb[C1:128, dj, :],
                                 H[C1:128, tB + 1:tB + 3, dj:dj + W2],
                                 start=False, stop=stop)
            else:
                nc.tensor.matmul(PA[:], w2s_sb[C1:128, dj, :],
                                 H[C1:128, tA + 1:tA + 3, dj:dj + W2],
                                 start=False, stop=stop)
                nc.tensor.matmul(PB[:], w2s_sb[0:C1, dj, :],
                                 H[0:C1, tB + 2:tB + 4, dj:dj + W2],
                                 start=False, stop=stop)
        # split the PSUM->SBUF copies across both PSUM-capable engines; all
        # out-DMAs stay on the gpsimd queue (sync queue must stay clear for
        # im2col prefetch -- a waiting out-DMA head-of-line blocks it)
        for t, P, copy_eng in ((tA, PA, nc.vector.tensor_copy),
                               (tB, PB, nc.scalar.copy)):
            O2 = opool.tile([C2, 2, W2], f32, tag="o2")
            copy_eng(O2[:], P[:])
            nc.gpsimd.dma_start(out[n, :, y0 + t:y0 + t + 2, :], O2[:])
        if mid_hook is not None:
            mid_hook(3)

    strips = [(n, y0) for n in range(g.npc) for y0 in range(0, g.h2, TY)]
    n_c1 = (TY + 2 + 1) // 2          # conv1 slots per strip (r = 0,2..TY)
    # software pipeline, im2col prefetched a full strip early:
    #   strip s body: issue B1(s+2) DMAs, conv2(s) interleaved with conv1(s+1)
    # so B1(s+1) had all of strip s-1's compute (~17us) to stream in.
    B1 = {0: load_b1(*strips[0], spread=True)}
    # conv2 weights can land any time before conv2(0); off the critical queue
    nc.gpsimd.dma_start(w2p_sb[:], w2p)
    nc.gpsimd.dma_start(w2s_sb[:], w2s2)
    if len(strips) > 1:
        B1[1] = load_b1(*strips[1], spread=True)
    Hcur = alloc_h()
    for r in range(0, TY + 2, 2):
        eng = nc.vector.tensor_copy if (r // 2) % 2 else nc.scalar.copy
        conv1_slot(B1[0], Hcur, r, eng)
    for i, (n, y0) in enumerate(strips):
        nxt = strips[i + 1] if i + 1 < len(strips) else None
        if i + 2 < len(strips):
            B1[i + 2] = load_b1(*strips[i + 2])
        if nxt is not None:
            Hnxt = alloc_h()
        state = {"r": 0, "h": 0}
        n_hooks = 3 * (TY // 4)       # 3 hook points per tile-pair

        def c1_hook(_phase):
            # rate-controlled even spread of the n_c1 conv1 slots over the
            # strip's hook points -- bursts starve ps1 banks + copy engines
            state["h"] += 1
            target = (n_c1 * state["h"] + n_hooks - 1) // n_hooks
            while state["r"] < min(target, n_c1):
                rr = state["r"]
                eng = nc.vector.tensor_copy if rr % 2 else nc.scalar.copy
                conv1_slot(B1[i + 1], Hnxt, 2 * rr, eng)
                state["r"] += 1

        for tA in range(0, TY, 4):
            hook = c1_hook if nxt is not None else None
            conv2_pair(n, y0, Hcur, tA, mid_hook=hook)
        B1.pop(i, None)
        if nxt is not None:
            Hcur = Hnxt


def build(g: Geom = GEOM, mm_dt=None):
    if mm_dt is None:
        mm_dt = _mm_dt()
    nc = bacc.Bacc("TRN2", target_bir_lowering=False, debug=False,
                   num_devices=N_CORES)
    f32 = mybir.dt.float32
    x = nc.dram_tensor("x", [g.npc, C0, g.h0, g.w0], mm_dt,
                       kind="ExternalInput").ap()
    w1t = nc.dram_tensor("w1t", [27, C1], mm_dt, kind="ExternalInput").ap()
    w2p = nc.dram_tensor("w2p", [128, 3, C2], mm_dt, kind="ExternalInput").ap()
    w2s2 = nc.dram_tensor("w2s2", [128, 3, C2], mm_dt,
                          kind="ExternalInput").ap()
    out = nc.dram_tensor("out", [g.npc, C2, g.h2, g.w2], f32,
                         kind="ExternalOutput").ap()
    with tile.TileContext(nc) as tc:
        with ExitStack() as ctx:
            _emit(ctx, tc, g, out, x, w1t, w2p, w2s2, mm_dt)
    nc.compile()
    return nc


def host_round(a: np.ndarray) -> np.ndarray:
    """Cast fp32 to the matmul storage dtype (bf16 cast, or tf32 rounding)."""
    a = np.ascontiguousarray(a, dtype=np.float32)
    if MODE == "bf16":
        return a.astype(ml_dtypes.bfloat16)
    b = a.view(np.uint32).copy()
    b += 0xFFF + ((b >> 13) & 1)
    b &= np.uint32(0xFFFFE000)
    return b.view(np.float32)


def pack_weights(w1: np.ndarray, w2: np.ndarray):
    """Host-side repack so every device DMA is contiguous.

    w1t[p, o] = w1[o, c, di, dj] with p = (di*3+dj)*3 + c  (matches im2col)
    w2p[k, dj, o]: k<64 -> w2[o, k, 0, dj]; k>=64 -> w2[o, k-64, 1, dj]
    w2s2[k, dj, o] = w2[o, k mod 64, 2, dj]  (tap-2 weights, both halves)
    """
    w1 = np.ascontiguousarray(np.asarray(w1), dtype=np.float32)
    w2 = np.ascontiguousarray(np.asarray(w2), dtype=np.float32)
    w1t = np.ascontiguousarray(w1.transpose(2, 3, 1, 0).reshape(27, C1))
    w2p = np.empty((128, 3, C2), np.float32)
    w2p[:C1] = w2[:, :, 0, :].transpose(1, 2, 0)
    w2p[C1:] = w2[:, :, 1, :].transpose(1, 2, 0)
    w2s = w2[:, :, 2, :].transpose(1, 2, 0)
    w2s2 = np.ascontiguousarray(np.concatenate([w2s, w2s], axis=0))
    return host_round(w1t), host_round(w2p), host_round(w2s2)


_NC_CACHE: dict = {}


def _get_nc():
    key = ("main", MODE)
    if key not in _NC_CACHE:
        _NC_CACHE[key] = build()
    return _NC_CACHE[key]


def run(x, w1, w2, trace: bool = False):
    """Shard, run on 8 cores, gather.  Returns (out, BassKernelResults)."""
    x = np.ascontiguousarray(np.asarray(x), dtype=np.float32)
    assert x.shape == (FULL_N, C0, GEOM.h0, GEOM.w0), x.shape
    w1t, w2p, w2s2 = pack_weights(w1, w2)
    xs = host_round(x).reshape(N_CORES, GEOM.npc, C0, GEOM.h0, GEOM.w0)
    in_maps = [
        {"x": np.ascontiguousarray(xs[c]), "w1t": w1t, "w2p": w2p,
         "w2s2": w2s2}
        for c in range(N_CORES)
    ]
    nc = _get_nc()
    res = bass_utils.run_bass_kernel_spmd(
        nc, in_maps, core_ids=list(range(N_CORES)), trace=trace)
    out = np.concatenate([r["out"] for r in res.results], axis=0)
    return out, res


def kernel(x, w1, w2):
    out, _ = run(x, w1, w2, trace=False)
    return out


# revision 20
# speedup vs baseline: 1.0937x; 1.0023x over previous
"""Trainium2 Bass/Tile kernel: two chained VALID 3x3 convolutions.

    x  [N,3,256,256] --conv(w1)--> h [N,64,254,254] --conv(w2)--> out [N,128,252,252]

Data-parallel over 8 NeuronCores: batch N=16 -> 2 images per core, conv
weights replicated.  Per core the convs are computed as implicit GEMMs on the
tensor engine (PE observed pinned at the 1.2 GHz throttled clock in this
environment, ~420 ns per 504-column bf16 matmul, so the win is fewer/denser
passes, not HAM warmup).

  conv1: contraction over C0*3*3=27 on SBUF partitions (im2col buffer built
         with 9 strided DMAs).  Column-tiled pair of matmuls per 2-row chunk
         produces the *doubled* h layout directly in PSUM:
           partitions 0:64  <- h rows (r, r+1)     (tile_position (0,0))
           partitions 64:128<- h rows (r+1, r+2)   (tile_position (0,64))
         so no SBUF->SBUF row-shift DMA is needed.

  conv2: contraction over C1*9=576 = 4.5 x 128.  Per output row-pair tile:
         3 K=128 matmuls cover taps (0,dj)+(1,dj) using the doubled H.
         The leftover taps (2,dj) are K=64 singles; singles of TWO adjacent
         output tiles are row-group-packed (tile_position rows 0 vs 64) so
         they run concurrently in the PE array: 9 effective passes per two
         tiles instead of 12.  PSUM accumulates; DVE copies to SBUF; DMA out.

MODE "bf16": inputs cast to bfloat16 host-side, fp32 PSUM accumulation
(measured scale-rel absmax err ~3.5e-3).
"""

from contextlib import ExitStack

import ml_dtypes
import numpy as np

import concourse.bass as bass
import concourse.mybir as mybir
import concourse.tile as tile
import concourse.bass_utils as bass_utils
from concourse import bacc

N_CORES = 8
FULL_N = 16
C0, C1, C2 = 3, 64, 128

MODE = "bf16"


def _mm_dt():
    return mybir.dt.bfloat16 if MODE == "bf16" else mybir.dt.float32r


def _np_dt():
    return ml_dtypes.bfloat16 if MODE == "bf16" else np.float32


class Geom:
    def __init__(self, npc, h0, w0, ty):
        self.npc = npc          # images per core
        self.h0, self.w0 = h0, w0
        self.h1, self.w1 = h0 - 2, w0 - 2
        self.h2, self.w2 = h0 - 4, w0 - 4
        self.ty = ty            # conv2 output rows per strip
        assert ty % 4 == 0 and self.h2 % ty == 0


GEOM = Geom(npc=FULL_N // N_CORES, h0=256, w0=256, ty=28)


def _emit(ctx: ExitStack, tc: tile.TileContext, g: Geom, out, x, w1t, w2p, w2s2,
          mm_dt):
    nc = tc.nc
    f32 = mybir.dt.float32
    TY, W1, W2 = g.ty, g.w1, g.w2

    wpool = ctx.enter_context(tc.tile_pool(name="weights", bufs=1))
    b1pool = ctx.enter_context(tc.tile_pool(name="b1", bufs=3))
    # bufs=3: H(s+1) must reuse a buffer whose readers (conv2 two strips
    # back) are long done, else the scheduler statically defers all of
    # conv1(s+1) to the end of strip s (burst -> copy backlog -> PE stall)
    hpool = ctx.enter_context(tc.tile_pool(name="h", bufs=3))
    opool = ctx.enter_context(tc.tile_pool(name="o2", bufs=4))
    ps1 = ctx.enter_context(tc.tile_pool(name="ps1", bufs=4, space="PSUM"))
    ps2 = ctx.enter_context(tc.tile_pool(name="ps2", bufs=4, space="PSUM"))

    w1t_sb = wpool.tile([27, C1], mm_dt)
    nc.sync.dma_start(w1t_sb[:], w1t)
    w2p_sb = wpool.tile([128, 3, C2], mm_dt)
    w2s_sb = wpool.tile([128, 3, C2], mm_dt)

    def load_b1(n, y0, spread=False):
        """Issue the 9 im2col DMAs for strip (n, y0); returns the tile."""
        # partition (di*3+dj)*3+c holds x[c, y0+r+di, dj:dj+W1]
        B1 = b1pool.tile([27, TY + 2, W1], mm_dt, tag="b1")
        engs = (nc.sync, nc.gpsimd, nc.scalar) if spread else (nc.sync,)
        for t9 in range(9):
            di, dj = divmod(t9, 3)
            engs[t9 % len(engs)].dma_start(
                B1[3 * t9:3 * t9 + 3],
                x[n, :, y0 + di:y0 + di + TY + 2, dj:dj + W1])
        return B1

    def alloc_h():
        return hpool.tile([128, TY + 2, W1], mm_dt, tag="h", name="H")

    def conv1_slot(B1, H, r, copy_eng):
        """Produce doubled h rows: top = (r, r+1), bottom = (r+1, r+2)."""
        last = r + 2 >= TY + 2  # bottom half would run past the strip
        P1 = ps1.tile([128, 2, W1], f32, tag="p1")
        # col group 0: h rows r, r+1 -> PSUM partitions 0:64
        nc.tensor.matmul(P1[0:C1], w1t_sb[:], B1[:, r:r + 2, :],
                         start=True, stop=True)
        if not last:
            # col group 64: h rows r+1, r+2 -> PSUM partitions 64:128
            nc.tensor.matmul(P1[C1:128], w1t_sb[:], B1[:, r + 1:r + 3, :],
                             start=True, stop=True)
            copy_eng(H[:, r:r + 2, :], P1[:])
        else:
            copy_eng(H[0:C1, r:r + 2, :], P1[0:C1])

    def conv2_pair(n, y0, H, tA, mid_hook=None):
        """Two output row-pair tiles (tA, tB=tA+2): 6 K=128 pair matmuls +
        6 K=64 singles packed two-at-a-time into PE row groups 0/64.
        mid_hook() is called between the pair blocks to interleave conv1
        slots finely (keeps copy engines evenly loaded, PE dense)."""
        tB = tA + 2
        PA = ps2.tile([C2, 2, W2], f32, tag="p2")
        PB = ps2.tile([C2, 2, W2], f32, tag="p2")
        for dj in range(3):  # taps (0,dj)+(1,dj) for tile A
            nc.tensor.matmul(PA[:], w2p_sb[:, dj, :],
                             H[:, tA:tA + 2, dj:dj + W2],
                             start=(dj == 0), stop=False)
        if mid_hook is not None:
            mid_hook(1)
        for dj in range(3):  # taps (0,dj)+(1,dj) for tile B
            nc.tensor.matmul(PB[:], w2p_sb[:, dj, :],
                             H[:, tB:tB + 2, dj:dj + W2],
                             start=(dj == 0), stop=False)
        if mid_hook is not None:
            mid_hook(2)
        # singles: tap (2,dj).  Top half (rows r = h row r) serves tile A,
        # bottom half (rows r = h row r+1) serves tile B, concurrently.
        # Bottom rows only span 0..TY-1, so the last tile-pair flips:
        # B reads top (needs h row TY+1), A reads bottom.
        b_on_top = tB + 2 > TY - 1  # B's bottom rows tB+1..tB+2 out of range
        for dj in range(3):
            stop = dj == 2
            if not b_on_top:
                nc.tensor.matmul(PA[:], w2s_sb[0:C1, dj, :],
                                 H[0:C1, tA + 2:tA + 4, dj:dj + W2],
                                 start=False, stop=stop)
                nc.tensor.matmul(PB[:], w2s_sb[C1:128, dj, :],
                                 H[C1:128, tB + 1:tB + 3, dj:dj + W2],
                                 start=False, stop=stop)
            else:
                nc.tensor.matmul(PA[:], w2s_sb[C1:128, dj, :],
                                 H[C1:128, tA + 1:tA + 3, dj:dj + W2],
                                 start=False, stop=stop)
                nc.tensor.matmul(PB[:], w2s_sb[0:C1, dj, :],
                                 H[0:C1, tB + 2:tB + 4, dj:dj + W2],
                                 start=False, stop=stop)
        # split the PSUM->SBUF copies across both PSUM-capable engines; all
        # out-DMAs stay on the gpsimd queue (sync queue must stay clear for
        # im2col prefetch -- a waiting out-DMA head-of-line blocks it)
        for t, P, copy_eng in ((tA, PA, nc.vector.tensor_copy),
                               (tB, PB, nc.scalar.copy)):
            O2 = opool.tile([C2, 2, W2], f32, tag="o2")
            copy_eng(O2[:], P[:])
            nc.gpsimd.dma_start(out[n, :, y0 + t:y0 + t + 2, :], O2[:])
        if mid_hook is not None:
            mid_hook(3)

    strips = [(n, y0) for n in range(g.npc) for y0 in range(0, g.h2, TY)]
    n_c1 = (TY + 2 + 1) // 2          # conv1 slots per strip (r = 0,2..TY)
    # software pipeline, im2col prefetched a full strip early:
    #   strip s body: issue B1(s+2) DMAs, conv2(s) interleaved with conv1(s+1)
    # so B1(s+1) had all of strip s-1's compute (~17us) to stream in.
    B1 = {0: load_b1(*strips[0], spread=True)}
    # conv2 weights can land any time before conv2(0); off the critical queue
    nc.gpsimd.dma_start(w2p_sb[:], w2p)
    nc.gpsimd.dma_start(w2s_sb[:], w2s2)
    if len(strips) > 1:
        B1[1] = load_b1(*strips[1], spread=True)
    Hcur = alloc_h()
    for r in range(0, TY + 2, 2):
        eng = nc.vector.tensor_copy if (r // 2) % 2 else nc.scalar.copy
        conv1_slot(B1[0], Hcur, r, eng)
    for i, (n, y0) in enumerate(strips):
        nxt = strips[i + 1] if i + 1 < len(strips) else None
        if i + 2 < len(strips):
            B1[i + 2] = load_b1(*strips[i + 2])
        if nxt is not None:
            Hnxt = alloc_h()
        state = {"r": 0, "h": 0}
        n_hooks = 3 * (TY // 4)       # 3 hook points per tile-pair

        def c1_hook(_phase):
            # rate-controlled even spread of the n_c1 conv1 slots over the
            # strip's hook points -- bursts starve ps1 banks + copy engines
            state["h"] += 1
            target = (n_c1 * state["h"] + n_hooks - 1) // n_hooks
            while state["r"] < min(target, n_c1):
                rr = state["r"]
                eng = nc.vector.tensor_copy if rr % 2 else nc.scalar.copy
                conv1_slot(B1[i + 1], Hnxt, 2 * rr, eng)
                state["r"] += 1

        for tA in range(0, TY, 4):
            hook = c1_hook if nxt is not None else None
            conv2_pair(n, y0, Hcur, tA, mid_hook=hook)
        B1.pop(i, None)
        if nxt is not None:
            Hcur = Hnxt


def build(g: Geom = GEOM, mm_dt=None):
    if mm_dt is None:
        mm_dt = _mm_dt()
    nc = bacc.Bacc("TRN2", target_bir_lowering=False, debug=False,
                   num_devices=N_CORES)
    f32 = mybir.dt.float32
    x = nc.dram_tensor("x", [g.npc, C0, g.h0, g.w0], mm_dt,
                       kind="ExternalInput").ap()
    w1t = nc.dram_tensor("w1t", [27, C1], mm_dt, kind="ExternalInput").ap()
    w2p = nc.dram_tensor("w2p", [128, 3, C2], mm_dt, kind="ExternalInput").ap()
    w2s2 = nc.dram_tensor("w2s2", [128, 3, C2], mm_dt,
                          kind="ExternalInput").ap()
    out = nc.dram_tensor("out", [g.npc, C2, g.h2, g.w2], f32,
                         kind="ExternalOutput").ap()
    with tile.TileContext(nc) as tc:
        with ExitStack() as ctx:
            _emit(ctx, tc, g, out, x, w1t, w2p, w2s2, mm_dt)
    nc.compile()
    return nc


def host_round(a: np.ndarray) -> np.ndarray:
    """Cast fp32 to the matmul storage dtype (bf16 cast, or tf32 rounding)."""
    a = np.ascontiguousarray(a, dtype=np.float32)
    if MODE == "bf16":
        return a.astype(ml_dtypes.bfloat16)
    b = a.view(np.uint32).copy()
    b += 0xFFF + ((b >> 13) & 1)
    b &= np.uint32(0xFFFFE000)
    return b.view(np.float32)


def pack_weights(w1: np.ndarray, w2: np.ndarray):
    """Host-side repack so every device DMA is contiguous.

    w1t[p, o] = w1[o, c, di, dj] with p = (di*3+dj)*3 + c  (matches im2col)
    w2p[k, dj, o]: k<64 -> w2[o, k, 0, dj]; k>=64 -> w2[o, k-64, 1, dj]
    w2s2[k, dj, o] = w2[o, k mod 64, 2, dj]  (tap-2 weights, both halves)
    """
    w1 = np.ascontiguousarray(np.asarray(w1), dtype=np.float32)
    w2 = np.ascontiguousarray(np.asarray(w2), dtype=np.float32)
    w1t = np.ascontiguousarray(w1.transpose(2, 3, 1, 0).reshape(27, C1))
    w2p = np.empty((128, 3, C2), np.float32)
    w2p[:C1] = w2[:, :, 0, :].transpose(1, 2, 0)
    w2p[C1:] = w2[:, :, 1, :].transpose(1, 2, 0)
    w2s = w2[:, :, 2, :].transpose(1, 2, 0)
    w2s2 = np.ascontiguousarray(np.concatenate([w2s, w2s], axis=0))
    return host_round(w1t), host_round(w2p), host_round(w2s2)


_NC_CACHE: dict = {}


def _get_nc():
    key = ("main", MODE)
    if key not in _NC_CACHE:
        _NC_CACHE[key] = build()
    return _NC_CACHE[key]


def run(x, w1, w2, trace: bool = False):
    """Shard, run on 8 cores, gather.  Returns (out, BassKernelResults)."""
    x = np.ascontiguousarray(np.asarray(x), dtype=np.float32)
    assert x.shape == (FULL_N, C0, GEOM.h0, GEOM.w0), x.shape
    w1t, w2p, w2s2 = pack_weights(w1, w2)
    xs = host_round(x).reshape(N_CORES, GEOM.npc, C0, GEOM.h0, GEOM.w0)
    in_maps = [
        {"x": np.ascontiguousarray(xs[c]), "w1t": w1t, "w2p": w2p,
         "w2s2": w2s2}
        for c in range(N_CORES)
    ]
    nc = _get_nc()
    res = bass_utils.run_bass_kernel_spmd(
        nc, in_maps, core_ids=list(range(N_CORES)), trace=trace)
    out = np.concatenate([r["out"] for r in res.results], axis=0)
    return out, res


def kernel(x, w1, w2):
    out, _ = run(x, w1, w2, trace=False)
    return out
